# revision 5
# baseline (speedup 1.0000x reference)
"""Trainium2 Bass kernel for nn_BaseSingleSplitDNAMiteModel (DNAMite: per-feature
and per-pair tiny MLPs over embedded categorical inputs, gated by smooth-z).

v2 strategy (8 NeuronCores, pair-sharded: 62 pairs/core, full batch per core):
  - Mains are exact 64-entry tables per feature -> computed on host in f32
    (0.8% of model FLOPs) and added to the device pair sum.
  - Pairs: host gathers the two embedding vectors per (batch, pair) into a
    dense rhs [128, CB] per 2-pair group: rows = [e_i(p0); e_j(p0); e_i(p1);
    e_j(p1)] (32 each).  Layer0 = ONE K=128 matmul per group with
    block-diagonal weights (both pairs in one 512-col stream).
  - Layer1: ONE K=128 matmul per group, block-diag(pw1[p0], pw1[p1]).
  - Layer2: ONE K=128 matmul per group (gate z folded into w2), PSUM-
    accumulated across groups into 4 rotating accumulator rows.
  - ReLU+bias ride the mandatory PSUM->SBUF copies, alternating DVE/ACT.
  - repeat>1 runs the body in a tc.For_i hardware loop (constant program
    size) so wall-clock slope between two repeat variants isolates true
    per-iteration HW exec time.
"""

import sys
from contextlib import ExitStack

import numpy as np

if "/opt/trn_rl_repo" not in sys.path:
    sys.path.insert(0, "/opt/trn_rl_repo")

import ml_dtypes

import concourse.bass as bass
import concourse.tile as tile
from concourse import bacc, mybir
from concourse import bass_utils

dt = mybir.dt
BF16 = ml_dtypes.bfloat16

# Model constants (hardcoded per the problem spec)
N_CORES = 8
B = 2048
F = 32          # features
E = 32          # embed dim
H = 64          # hidden
FS = 64         # feature size (vocab per feature)
P = 496         # pairs
GAMMA = 1.0
CB = 512        # batch chunk processed per wave
NPSHARD = 4     # pair shards
NBSHARD = 2     # batch shards (core = bshard*NPSHARD + pshard)
BL = B // NBSHARD       # 1024 batch rows per core
NCHUNK = BL // CB       # 2
PL = P // NPSHARD       # 124 pairs per core
NPG = PL // 2           # 62 pair groups of 2

RELU = mybir.ActivationFunctionType.Relu
ADD = mybir.AluOpType.add
MAX = mybir.AluOpType.max

UNROLL = 4      # bodies per For_i iteration in timing variants

_prog_cache = {}


def _smooth_z(z):
    s = -2.0 / GAMMA**3 * z**3 + 3.0 / (2.0 * GAMMA) * z + 0.5
    return np.where(z <= -GAMMA / 2, 0.0, np.where(z >= GAMMA / 2, 1.0, s)).astype(np.float32)


def _build_program(repeat=1):
    """One SPMD program; per-core data differs via in_maps."""
    nc = bacc.Bacc("TRN2", target_bir_lowering=False, debug=False, num_devices=N_CORES)

    def din(name, shape, dtype):
        return nc.dram_tensor(name, shape, dtype, kind="ExternalInput").ap()

    d_w0 = din("w0", (128, NPG * 128), dt.bfloat16)
    d_w1 = din("w1", (128, NPG * 128), dt.bfloat16)
    d_w2 = din("w2", (128, NPG), dt.bfloat16)
    d_b0 = din("b0", (128, NPG), dt.float32)
    d_b1 = din("b1", (128, NPG), dt.float32)
    d_rhs = din("rhs", (128, NCHUNK * NPG * CB), dt.bfloat16)
    d_out = nc.dram_tensor("out", (NCHUNK * 128, CB), dt.float32, kind="ExternalOutput").ap()

    relu_ctr = [0]

    with tile.TileContext(nc) as tc, ExitStack() as ctx:
        wres = ctx.enter_context(tc.tile_pool(name="wres", bufs=1))
        rhspool = ctx.enter_context(tc.tile_pool(name="rhsp", bufs=2))
        h0pool = ctx.enter_context(tc.tile_pool(name="h0", bufs=4))
        h1pool = ctx.enter_context(tc.tile_pool(name="h1", bufs=4))
        outpool = ctx.enter_context(tc.tile_pool(name="outp", bufs=2))
        import os as _os
        _pb = _os.environ.get("K_PSUM", "4,3,1")
        _b0, _b1, _ba = (int(x) for x in _pb.split(","))
        ps0 = ctx.enter_context(tc.tile_pool(name="ps0", bufs=_b0, space="PSUM"))
        ps1 = ctx.enter_context(tc.tile_pool(name="ps1", bufs=_b1, space="PSUM"))
        psacc = ctx.enter_context(tc.tile_pool(name="psacc", bufs=_ba, space="PSUM"))

        # --- resident loads ---
        sb_w0 = wres.tile([128, NPG * 128], dt.bfloat16, tag="w0")
        nc.sync.dma_start(sb_w0[:], d_w0)
        sb_w1 = wres.tile([128, NPG * 128], dt.bfloat16, tag="w1")
        nc.sync.dma_start(sb_w1[:], d_w1)
        sb_w2 = wres.tile([128, NPG], dt.bfloat16, tag="w2")
        nc.sync.dma_start(sb_w2[:], d_w2)
        sb_b0 = wres.tile([128, NPG], dt.float32, tag="b0")
        nc.sync.dma_start(sb_b0[:], d_b0)
        sb_b1 = wres.tile([128, NPG], dt.float32, tag="b1")
        nc.sync.dma_start(sb_b1[:], d_b1)

        def relu_copy(dst, src, bias_ap):
            """dst(bf16 sbuf) = relu(src(psum f32) + bias).

            HW-measured: DVE slightly faster -> DVE takes 63/124, ACT 61."""
            i = relu_ctr[0] % 124
            relu_ctr[0] += 1
            if i % 2 == 0 or i == 1:
                nc.vector.tensor_scalar(dst, src, bias_ap, 0.0, ADD, MAX)
            else:
                nc.scalar.activation(dst, src, RELU, bias=bias_ap)

        # rhs DMA is split so group-0 compute starts after ~1MB, not 4MB
        RHS_SPLIT = (8, 8, 8, 8, 8, 8, 8, 6)
        _resident = bool(_os.environ.get("K_RESIDENT"))
        _res_tiles = {}
        if _resident:
            for c in range(NCHUNK):
                tile_ = wres.tile([128, NPG * CB], dt.bfloat16, tag=f"rhsres{c}")
                nc.sync.dma_start(tile_[:], d_rhs[:, c * NPG * CB:(c + 1) * NPG * CB])
                _res_tiles[c] = tile_

        def chunk(c):
            if _resident:
                def rhs_of(g, _t=_res_tiles[c]):
                    return _t[:, g * CB:(g + 1) * CB]
            else:
                sb_rhs = []
                g0 = 0
                for si, glen in enumerate(RHS_SPLIT):
                    tile_ = rhspool.tile([128, glen * CB], dt.bfloat16, tag=f"rhs{si}")
                    off = c * NPG * CB + g0 * CB
                    nc.sync.dma_start(tile_[:], d_rhs[:, off:off + glen * CB])
                    sb_rhs.append((g0, tile_))
                    g0 += glen

                def rhs_of(g):
                    for (gs, tile_), glen in zip(sb_rhs, RHS_SPLIT):
                        if gs <= g < gs + glen:
                            return tile_[:, (g - gs) * CB:(g - gs + 1) * CB]
                    raise AssertionError

            acc = psacc.tile([128, CB], dt.float32, tag="acc")
            # l2 accumulates group g into PSUM row 32*(g%4); track start/stop
            slot_first = {0: True, 32: True, 64: True, 96: True}
            n_hits = {0: 0, 32: 0, 64: 0, 96: 0}
            for g in range(NPG):
                n_hits[32 * (g % 4)] += 1

            for g in range(NPG):
                ps0t = ps0.tile([128, CB], dt.float32, tag="l0")
                nc.tensor.matmul(ps0t[:], sb_w0[:, g * 128:(g + 1) * 128],
                                 rhs_of(g),
                                 start=True, stop=True)
                h0 = h0pool.tile([128, CB], dt.bfloat16, tag="h0")
                relu_copy(h0[:], ps0t[:], sb_b0[:, g:g + 1])

                ps1t = ps1.tile([128, CB], dt.float32, tag="l1")
                nc.tensor.matmul(ps1t[:], sb_w1[:, g * 128:(g + 1) * 128], h0[:],
                                 start=True, stop=True)
                h1 = h1pool.tile([128, CB], dt.bfloat16, tag="h1")
                relu_copy(h1[:], ps1t[:], sb_b1[:, g:g + 1])

                slot = 32 * (g % 4)
                st = slot_first[slot]
                slot_first[slot] = False
                n_hits[slot] -= 1
                nc.tensor.matmul(acc[slot:slot + 1, :], sb_w2[:, g:g + 1], h1[:],
                                 start=st, stop=(n_hits[slot] == 0),
                                 tile_position=(0, slot), skip_group_check=True)

            # ---- drain accumulators (ACT; DVE is the busier engine) ----
            outsb = outpool.tile([128, CB], dt.float32, tag="outsb")
            nc.scalar.activation(outsb[:], acc[:], mybir.ActivationFunctionType.Copy)
            nc.sync.dma_start(d_out[c * 128:(c + 1) * 128, :], outsb[:])

        def body():
            for c in range(NCHUNK):
                chunk(c)

        if repeat == 1:
            body()
        else:
            # UNROLL bodies per loop iteration: amortizes the For_i
            # all-engine barrier and lets body N+1's first DMA overlap
            # body N's tail, so the slope measures steady-state
            # per-body throughput.
            assert repeat % UNROLL == 0
            with tc.For_i(0, repeat // UNROLL):
                for _ in range(UNROLL):
                    body()

    nc.compile()
    return nc


def _pack_core(ci, E_rhs, pw0q, pw1q, w2q, pb0, pb1):
    """Build the per-core in_map.

    E_rhs: [8, 128, NCHUNK*NPG*CB] bf16 (prebuilt for all cores)
    pw0q/pw1q: [P, 64, 64] bf16; w2q: [P, 64] bf16 (gate folded);
    pb0/pb1: [P, 64] f32.
    """
    sl = slice((ci % NPSHARD) * PL, (ci % NPSHARD + 1) * PL)

    w0 = np.zeros((128, NPG, 128), BF16)
    w0[0:64, :, 0:64] = pw0q[sl][0::2].transpose(1, 0, 2)
    w0[64:128, :, 64:128] = pw0q[sl][1::2].transpose(1, 0, 2)

    w1 = np.zeros((128, NPG, 128), BF16)
    w1[0:64, :, 0:64] = pw1q[sl][0::2].transpose(1, 0, 2)
    w1[64:128, :, 64:128] = pw1q[sl][1::2].transpose(1, 0, 2)

    w2 = np.empty((128, NPG), BF16)
    w2[0:64] = w2q[sl][0::2].T
    w2[64:128] = w2q[sl][1::2].T

    b0 = np.empty((128, NPG), np.float32)
    b0[0:64] = pb0[sl][0::2].T
    b0[64:128] = pb0[sl][1::2].T
    b1 = np.empty((128, NPG), np.float32)
    b1[0:64] = pb1[sl][0::2].T
    b1[64:128] = pb1[sl][1::2].T

    return {
        "w0": np.ascontiguousarray(w0.reshape(128, NPG * 128)),
        "w1": np.ascontiguousarray(w1.reshape(128, NPG * 128)),
        "w2": w2,
        "b0": b0,
        "b1": b1,
        "rhs": E_rhs[ci],
    }


def kernel(**inputs):
    inp = {k: np.asarray(v) for k, v in inputs.items()}
    mains_i = inp["mains"].astype(np.int64)
    pairs_i = inp["pairs"].astype(np.int64)
    pairs_list = inp["pairs_list"].astype(np.int64)
    emb2 = inp["embedding"].astype(np.float32)          # [F*FS, E]
    emb = emb2.reshape(F, FS, E)

    zs_m = _smooth_z(inp["z_main"].astype(np.float32))
    zs_p = _smooth_z(inp["z_pairs"].astype(np.float32))

    # ---- mains: exact per-feature 64-entry tables on host (f32) ----
    t = np.einsum("fve,feh->fvh", emb, inp["mw0"].astype(np.float32)) + inp["mb0"][:, None, :]
    t = np.maximum(t, 0.0)
    t = np.einsum("fvh,fhg->fvg", t, inp["mw1"].astype(np.float32)) + inp["mb1"][:, None, :]
    t = np.maximum(t, 0.0)
    t = np.einsum("fvh,fho->fvo", t, inp["mw2"].astype(np.float32))[:, :, 0] + inp["mb2"][:, 0:1]
    tmain = t * zs_m[:, None]                            # [F, FS]
    out_main = np.take_along_axis(tmain, mains_i.T, axis=1).sum(axis=0)  # [B]

    # ---- pairs: device tensors ----
    pw0q = inp["pw0"].astype(BF16).reshape(P, 2 * E, H)  # [P,64,64]
    pw1q = inp["pw1"].astype(BF16)
    w2q = (inp["pw2"][:, :, 0] * zs_p[:, None]).astype(BF16)
    Cconst = float(np.dot(inp["pb2"][:, 0], zs_p))

    # gathered embedding rhs: [B, P, 2, E]
    idx = inp["offsets"].astype(np.int64)[pairs_list][None, :, :] + pairs_i  # [B,P,2]
    Eg = emb2[idx]                                       # [B,P,2,32] f32
    # -> [core=(bshard,pshard), row=(pairidx,side,e), chunk, group, cb]
    Eg = Eg.reshape(NBSHARD, NCHUNK, CB, NPSHARD, NPG, 2, 2, E)
    Eg = Eg.transpose(0, 3, 5, 6, 7, 1, 4, 2)            # [2, 4, 2, 2, 32, NCHUNK, NPG, CB]
    E_rhs = np.ascontiguousarray(Eg.astype(BF16)).reshape(N_CORES, 128, NCHUNK * NPG * CB)

    nc = _get_program(1)

    in_maps = [
        _pack_core(ci, E_rhs, pw0q, pw1q, w2q,
                   inp["pb0"].astype(np.float32), inp["pb1"].astype(np.float32))
        for ci in range(N_CORES)
    ]

    res = bass_utils.run_bass_kernel_spmd(nc, in_maps, core_ids=list(range(N_CORES)))
    globals()["_last_results"] = res
    globals()["_last_in_maps"] = in_maps

    out = np.zeros(B, dtype=np.float32)
    for ci in range(N_CORES):
        o = res.results[ci]["out"].reshape(NCHUNK, 128, CB)
        bs = ci // NPSHARD
        out[bs * BL:(bs + 1) * BL] += o[:, [0, 32, 64, 96], :].sum(axis=1).reshape(BL)
    out += out_main + Cconst
    return out[:, None].astype(np.float32)


def _get_program(repeat):
    if repeat not in _prog_cache:
        _prog_cache[repeat] = _build_program(repeat)
    return _prog_cache[repeat]


def bench(in_maps, repeat=1, iters=5):
    """Return per-call wall times (s) for the repeat-variant program."""
    import time
    nc = _get_program(repeat)
    times = []
    for _ in range(iters):
        t0 = time.time()
        bass_utils.run_bass_kernel_spmd(nc, in_maps, core_ids=list(range(N_CORES)))
        times.append(time.time() - t0)
    return times
